# revision 1
# baseline (speedup 1.0000x reference)
"""Trainium2 Bass kernel for nn_CrossPairMemory.

Sharding: data-parallel over batch across 8 NeuronCores (512 rows each),
weights replicated per core, no collectives.  All heavy matmuls run in
bf16 (fp32 PSUM accumulation); LayerNorm statistics and normalization in
fp32.  Activations are kept transposed (features on partitions, batch on
the free axis) through the fusion MLP so weight tiles act as the
stationary matmul operand in their natural HBM layout; the final
per-pair stage flips to activations-stationary so the output psum is
row-major and the last LayerNorm reduces along the free axis.
"""

import sys

for _p in ("/opt/trn_rl_repo",):
    if _p not in sys.path:
        sys.path.insert(0, _p)

import numpy as np
import ml_dtypes

import concourse.bass as bass
import concourse.tile as tile
from concourse import bacc, mybir
from concourse import bass_utils

BF = ml_dtypes.bfloat16
dt = mybir.dt
AF = mybir.ActivationFunctionType
ALU = mybir.AluOpType

NCORES = 8
B, P, PD, MD, S = 4096, 28, 128, 256, 64
D = P * PD            # 3584
K1T = 2 * P           # 56 contraction tiles for the first fusion matmul
Bc = B // NCORES      # 512 batch rows per core
# batch sub-chunks inside a core: small first chunk so its LN/gelu pass
# overlaps the second chunk's matmuls on the PE.
CHUNKS = ((0, 128), (128, 384))
EPS = 1e-5


def _bcast_ap(src_row):
    """Replicate a [N]-shaped dram AP across 128 partitions (stride-0)."""
    return bass.AP(
        tensor=src_row.tensor,
        offset=src_row.offset,
        ap=[[0, PD]] + [list(x) for x in src_row.ap],
    )


def _build():
    nc = bacc.Bacc(
        "TRN2", target_bir_lowering=False, debug=False, num_devices=NCORES
    )

    def din(name, shape, dty):
        return nc.dram_tensor(name, list(shape), dty, kind="ExternalInput").ap()

    psT = din("psT", (P, PD, Bc), dt.bfloat16)      # pair_states^T per pair
    msT = din("msT", (MD, Bc), dt.bfloat16)         # macro_state^T
    kP = din("kP", (PD, S), dt.bfloat16)            # pair keys^T, pre-scaled
    kM = din("kM", (MD, S), dt.bfloat16)            # macro keys^T, pre-scaled
    vP = din("vP", (S, D), dt.bfloat16)
    vM = din("vM", (S, D), dt.bfloat16)
    w1r = din("w1r", (P, PD, K1T, PD), dt.bfloat16)  # [n, kp, kt, f]
    w2r = din("w2r", (P, PD, P, PD), dt.bfloat16)    # [m, kp, kt, f]
    b1t = din("b1t", (PD, P), dt.float32)
    g1t = din("g1t", (PD, P), dt.float32)
    be1t = din("be1t", (PD, P), dt.float32)
    b2t = din("b2t", (PD, P), dt.float32)
    pwr = din("pwr", (PD, P, 2, PD), dt.bfloat16)    # [d, pair, ktile, e]
    pbr = din("pbr", (1, P, PD), dt.bfloat16)
    pgbc = din("pgbc", (PD, P, PD), dt.float32)      # ln_g broadcast rows
    pbbc = din("pbbc", (PD, P, PD), dt.float32)      # ln_b broadcast rows
    # output in [pair, btile, 128, 128] scratch layout: every DMA write is
    # one contiguous 64KB block; the host reassembles to (Bc, P, PD).
    out = nc.dram_tensor(
        "out", [P, Bc // PD, PD, PD], dt.float32, kind="ExternalOutput"
    ).ap()

    with tile.TileContext(nc) as tc:
        with (
            tc.tile_pool(name="const", bufs=1) as const,
            tc.tile_pool(name="res", bufs=1) as res,
        ):
            ones_col = const.tile([PD, 1], dt.bfloat16, tag="ones_col", name="ones_col")
            nc.vector.memset(ones_col, 1.0)
            ones_row_f = const.tile([1, PD], dt.float32, tag="ones_row_f", name="ones_row_f")
            nc.vector.memset(ones_row_f, 1.0)
            ones_row_b = const.tile([1, PD], dt.bfloat16, tag="ones_row_b", name="ones_row_b")
            nc.vector.memset(ones_row_b, 1.0)
            eps_t = const.tile([PD, 1], dt.float32, tag="eps", name="eps")
            nc.vector.memset(eps_t, EPS)

            lnc = {}
            for nm, src in (("b1", b1t), ("g1", g1t), ("be1", be1t), ("b2", b2t)):
                t = const.tile([PD, P], dt.float32, tag=f"lnc_{nm}", name=f"lnc_{nm}")
                nc.sync.dma_start(t, src)
                lnc[nm] = t
            pw_sb = const.tile([PD, P, 2, PD], dt.bfloat16, tag="pw_sb", name="pw_sb")
            nc.sync.dma_start(pw_sb, pwr)
            pb_sb = const.tile([1, P, PD], dt.bfloat16, tag="pb_sb", name="pb_sb")
            nc.sync.dma_start(pb_sb, pbr)

            # pair_states^T tiles stay resident: used by the score matmuls
            # (stage A) and again as stationary operands in stage C.
            psT_sb = []
            for p in range(P):
                t = res.tile([PD, Bc], dt.bfloat16, tag=f"psT{p}", name=f"psT{p}")
                nc.sync.dma_start(t, psT[p])
                psT_sb.append(t)

            with (
                tc.tile_pool(name="xt", bufs=1) as pxt,
                tc.tile_pool(name="h2", bufs=1) as ph2,
            ):
                xt_sb = [
                    pxt.tile([PD, Bc], dt.bfloat16, tag=f"xt{k}", name=f"xt{k}")
                    for k in range(K1T)
                ]
                h2_sb = [
                    ph2.tile([PD, Bc], dt.bfloat16, tag=f"h2{n}", name=f"h2{n}")
                    for n in range(P)
                ]

                # ---------------- stage A: associative memory reads --------
                with (
                    tc.tile_pool(name="stA", bufs=1) as pa,
                    tc.tile_pool(name="psA", bufs=2, space="PSUM") as ppa,
                    tc.tile_pool(name="psAc", bufs=2, space="PSUM") as ppac,
                ):
                    vP_sb = pa.tile([S, D], dt.bfloat16, tag="vP", name="vP")
                    nc.sync.dma_start(vP_sb, vP)
                    vM_sb = pa.tile([S, D], dt.bfloat16, tag="vM", name="vM")
                    nc.sync.dma_start(vM_sb, vM)
                    kP_sb = pa.tile([PD, S], dt.bfloat16, tag="kP", name="kP")
                    nc.sync.dma_start(kP_sb, kP)
                    kM0 = pa.tile([PD, S], dt.bfloat16, tag="kM0", name="kM0")
                    nc.sync.dma_start(kM0, kM[0:PD])
                    kM1 = pa.tile([PD, S], dt.bfloat16, tag="kM1", name="kM1")
                    nc.sync.dma_start(kM1, kM[PD:MD])
                    ms0 = pa.tile([PD, Bc], dt.bfloat16, tag="ms0", name="ms0")
                    nc.sync.dma_start(ms0, msT[0:PD])
                    ms1 = pa.tile([PD, Bc], dt.bfloat16, tag="ms1", name="ms1")
                    nc.sync.dma_start(ms1, msT[PD:MD])

                    def memory_read(which, vals_sb, xt_off):
                        sp = ppa.tile([S, Bc], dt.float32, tag="sp", name="sp")
                        if which == "pair":
                            for p in range(P):
                                nc.tensor.matmul(
                                    sp, kP_sb, psT_sb[p],
                                    start=(p == 0), stop=(p == P - 1),
                                )
                        else:
                            nc.tensor.matmul(sp, kM0, ms0, start=True, stop=False)
                            nc.tensor.matmul(sp, kM1, ms1, start=False, stop=True)
                        # scores are O(0.3): exp without max-subtraction is safe
                        eb = pa.tile([S, Bc], dt.bfloat16, tag=f"eb_{which}", name=f"eb_{which}")
                        nc.scalar.activation(eb, sp, AF.Exp)
                        den = ppa.tile([1, Bc], dt.float32, tag="den", name="den")
                        nc.tensor.matmul(den, ones_col[0:S, :], eb, start=True, stop=True)
                        rr = pa.tile([1, Bc], dt.float32, tag=f"rr_{which}", name=f"rr_{which}")
                        nc.vector.reciprocal(rr, den)
                        rbc = ppa.tile([S, Bc], dt.float32, tag="rbc", name="rbc")
                        nc.tensor.matmul(
                            rbc, ones_row_f[:, 0:S], rr, start=True, stop=True
                        )
                        ab = pa.tile([S, Bc], dt.bfloat16, tag=f"ab_{which}", name=f"ab_{which}")
                        nc.vector.tensor_mul(ab, eb, rbc)
                        for d in range(P):
                            pc = ppac.tile([PD, Bc], dt.float32, tag="pc", name="pc")
                            nc.tensor.matmul(
                                pc, vals_sb[:, d * PD:(d + 1) * PD], ab,
                                start=True, stop=True,
                            )
                            nc.scalar.activation(xt_sb[xt_off + d], pc, AF.Copy)

                    memory_read("pair", vP_sb, 0)
                    memory_read("macro", vM_sb, P)

                # ---------------- stage B: fusion MLP -----------------------
                with (
                    tc.tile_pool(name="hbf", bufs=1) as phb,
                    tc.tile_pool(name="psStat", bufs=1, space="PSUM") as ppst,
                ):
                    hbf = [
                        phb.tile([PD, Bc], dt.bfloat16, tag=f"hbf{n}", name=f"hbf{n}")
                        for n in range(P)
                    ]
                    stat_h = ppst.tile([1, Bc], dt.float32, tag="stat_h", name="stat_h")
                    stat_q = ppst.tile([1, Bc], dt.float32, tag="stat_q", name="stat_q")

                    with (
                        tc.tile_pool(name="w1s", bufs=2) as pw1,
                        tc.tile_pool(name="sqs", bufs=3) as psq,
                        tc.tile_pool(name="psM1", bufs=2, space="PSUM") as ppm1,
                    ):
                        for n in range(P):
                            w1b = pw1.tile([PD, K1T, PD], dt.bfloat16, tag="w1blk", name="w1blk")
                            nc.sync.dma_start(w1b, w1r[n])
                            for ci, (co, csz) in enumerate(CHUNKS):
                                pm = ppm1.tile([PD, csz], dt.float32, tag=f"pm{ci}", name=f"pm{ci}")
                                for k in range(K1T):
                                    nc.tensor.matmul(
                                        pm, w1b[:, k, :],
                                        xt_sb[k][:, co:co + csz],
                                        start=(k == 0), stop=(k == K1T - 1),
                                    )
                                nc.scalar.activation(
                                    hbf[n][:, co:co + csz], pm, AF.Identity,
                                    bias=lnc["b1"][:, n:n + 1], scale=1.0,
                                )
                            sq = psq.tile([PD, Bc], dt.bfloat16, tag="sq", name="sq")
                            nc.vector.tensor_mul(sq, hbf[n], hbf[n])
                            for co, csz in CHUNKS:
                                nc.tensor.matmul(
                                    stat_h[:, co:co + csz], ones_col,
                                    hbf[n][:, co:co + csz],
                                    start=(n == 0), stop=(n == P - 1),
                                    skip_group_check=True,
                                )
                                nc.tensor.matmul(
                                    stat_q[:, co:co + csz], ones_col,
                                    sq[:, co:co + csz],
                                    start=(n == 0), stop=(n == P - 1),
                                    skip_group_check=True,
                                )

                    # LayerNorm + gelu (per batch chunk)
                    with (
                        tc.tile_pool(name="lnrow", bufs=2) as plr,
                        tc.tile_pool(name="psBC", bufs=1, space="PSUM") as ppbc,
                        tc.tile_pool(name="tnorm", bufs=3) as ptn,
                    ):
                        for ci, (co, csz) in enumerate(CHUNKS):
                            cs = slice(co, co + csz)
                            mu_row = plr.tile([1, csz], dt.float32, tag=f"mu{ci}", name=f"mu{ci}")
                            nc.scalar.activation(
                                mu_row, stat_h[:, cs], AF.Copy, scale=1.0 / D
                            )
                            m2_row = plr.tile([1, csz], dt.float32, tag=f"m2{ci}", name=f"m2{ci}")
                            nc.scalar.activation(
                                m2_row, stat_q[:, cs], AF.Copy, scale=1.0 / D
                            )
                            var_row = plr.tile([1, csz], dt.float32, tag=f"va{ci}", name=f"va{ci}")
                            nc.vector.tensor_mul(var_row, mu_row, mu_row)
                            nc.vector.tensor_sub(var_row, m2_row, var_row)
                            sd_row = plr.tile([1, csz], dt.float32, tag=f"sd{ci}", name=f"sd{ci}")
                            nc.scalar.activation(
                                sd_row, var_row, AF.Sqrt,
                                bias=eps_t[0:1, :], scale=1.0,
                            )
                            rstd_row = plr.tile([1, csz], dt.float32, tag=f"rs{ci}", name=f"rs{ci}")
                            nc.vector.reciprocal(rstd_row, sd_row)
                            mu_bc = ppbc.tile([PD, csz], dt.float32, tag=f"mubc{ci}", name=f"mubc{ci}")
                            nc.tensor.matmul(
                                mu_bc, ones_row_f, mu_row, start=True, stop=True
                            )
                            rs_bc = ppbc.tile([PD, csz], dt.float32, tag=f"rsbc{ci}", name=f"rsbc{ci}")
                            nc.tensor.matmul(
                                rs_bc, ones_row_f, rstd_row, start=True, stop=True
                            )
                            for n in range(P):
                                t1 = ptn.tile([PD, csz], dt.float32, tag=f"t1_{ci}", name=f"t1_{ci}")
                                nc.vector.scalar_tensor_tensor(
                                    t1, hbf[n][:, cs], 1.0, mu_bc,
                                    op0=ALU.mult, op1=ALU.subtract,
                                )
                                t2 = ptn.tile([PD, csz], dt.float32, tag=f"t2_{ci}", name=f"t2_{ci}")
                                nc.vector.scalar_tensor_tensor(
                                    t2, t1, lnc["g1"][:, n:n + 1], rs_bc,
                                    op0=ALU.mult, op1=ALU.mult,
                                )
                                nc.scalar.activation(
                                    h2_sb[n][:, cs], t2, AF.Gelu,
                                    bias=lnc["be1"][:, n:n + 1], scale=1.0,
                                )

                # ------------- stage B2 + C: second matmul & per-pair -------
                with (
                    tc.tile_pool(name="w2s", bufs=2) as pw2,
                    tc.tile_pool(name="fus", bufs=3) as pfu,
                    tc.tile_pool(name="cbc", bufs=1) as pcb,
                    tc.tile_pool(name="scm", bufs=4) as psc,
                    tc.tile_pool(name="yout", bufs=3) as pyo,
                    tc.tile_pool(name="psM2", bufs=2, space="PSUM") as ppm2,
                    tc.tile_pool(name="psC", bufs=3, space="PSUM") as ppc,
                ):
                    gbc_sb = pcb.tile([PD, P, PD], dt.float32, tag="gbc_sb",
                                      name="gbc_sb")
                    nc.sync.dma_start(gbc_sb, pgbc)
                    bbc_sb = pcb.tile([PD, P, PD], dt.float32, tag="bbc_sb",
                                      name="bbc_sb")
                    nc.sync.dma_start(bbc_sb, pbbc)
                    for ci, (co, csz) in enumerate(CHUNKS):
                        cs = slice(co, co + csz)
                        for m in range(P):
                            w2b = pw2.tile([PD, P, PD], dt.bfloat16, tag="w2blk", name="w2blk")
                            nc.sync.dma_start(w2b, w2r[m])
                            pf = ppm2.tile([PD, csz], dt.float32, tag=f"pf{ci}", name=f"pf{ci}")
                            for k in range(P):
                                nc.tensor.matmul(
                                    pf, w2b[:, k, :], h2_sb[k][:, cs],
                                    start=(k == 0), stop=(k == P - 1),
                                )
                            fz = pfu.tile([PD, csz], dt.bfloat16, tag=f"fz{ci}", name=f"fz{ci}")
                            nc.scalar.activation(
                                fz, pf, AF.Identity,
                                bias=lnc["b2"][:, m:m + 1], scale=1.0,
                            )
                            gb = gbc_sb[:, m, :]
                            bb = bbc_sb[:, m, :]
                            for bt in range(csz // PD):
                                bs = slice(co + bt * PD, co + (bt + 1) * PD)
                                po = ppc.tile([PD, PD], dt.float32, tag="po", name="po")
                                nc.tensor.matmul(
                                    po, psT_sb[m][:, bs], pw_sb[:, m, 0, :],
                                    start=True, stop=False,
                                )
                                nc.tensor.matmul(
                                    po, fz[:, bt * PD:(bt + 1) * PD],
                                    pw_sb[:, m, 1, :],
                                    start=False, stop=False,
                                )
                                nc.tensor.matmul(
                                    po, ones_row_b, pb_sb[:, m, :],
                                    start=False, stop=True,
                                )
                                st6 = psc.tile([PD, 6], dt.float32, tag="st6", name="st6")
                                nc.vector.bn_stats(st6, po)
                                mv = psc.tile([PD, 2], dt.float32, tag="mv", name="mv")
                                nc.vector.bn_aggr(mv, st6)
                                sd2 = psc.tile([PD, 1], dt.float32, tag="sd2", name="sd2")
                                nc.scalar.activation(
                                    sd2, mv[:, 1:2], AF.Sqrt,
                                    bias=eps_t, scale=1.0,
                                )
                                rst2 = psc.tile([PD, 1], dt.float32, tag="rst2", name="rst2")
                                nc.vector.reciprocal(rst2, sd2)
                                tn = pyo.tile([PD, PD], dt.float32, tag="tn", name="tn")
                                nc.vector.tensor_scalar(
                                    tn, po, mv[:, 0:1], rst2,
                                    op0=ALU.subtract, op1=ALU.mult,
                                )
                                nc.vector.tensor_mul(tn, tn, gb)
                                y = pyo.tile([PD, PD], dt.float32, tag="y", name="y")
                                nc.vector.tensor_add(y, tn, bb)
                                nc.sync.dma_start(out[m, co // PD + bt], y)

    nc.compile()
    return nc


_CACHE = {}


def _get_nc():
    if "nc" not in _CACHE:
        _CACHE["nc"] = _build()
    return _CACHE["nc"]


def _prep_in_maps(inputs):
    f32 = np.float32
    g = lambda k: np.asarray(inputs[k], f32)

    psT_full = np.asarray(g("pair_states").transpose(1, 2, 0), dtype=BF)   # [P,PD,B]
    msT_full = np.asarray(g("macro_state").T, dtype=BF)                    # [MD,B]

    shared = {
        "kP": np.ascontiguousarray(
            (g("mem_pair_keys").T / (P * np.sqrt(PD))).astype(BF)),
        "kM": np.ascontiguousarray(
            (g("mem_macro_keys").T / np.sqrt(MD)).astype(BF)),
        "vP": g("mem_pair_vals").astype(BF),
        "vM": g("mem_macro_vals").astype(BF),
        "w1r": np.ascontiguousarray(
            g("fusion_w1").reshape(K1T, PD, P, PD).transpose(2, 1, 0, 3)
        ).astype(BF),
        "w2r": np.ascontiguousarray(
            g("fusion_w2").reshape(P, PD, P, PD).transpose(2, 1, 0, 3)
        ).astype(BF),
        "b1t": np.ascontiguousarray(g("fusion_b1").reshape(P, PD).T),
        "g1t": np.ascontiguousarray(g("fusion_ln_g").reshape(P, PD).T),
        "be1t": np.ascontiguousarray(g("fusion_ln_b").reshape(P, PD).T),
        "b2t": np.ascontiguousarray(g("fusion_b2").reshape(P, PD).T),
        "pwr": np.ascontiguousarray(
            g("pair_w").reshape(P, 2, PD, PD).transpose(2, 0, 1, 3)
        ).astype(BF),
        "pbr": g("pair_b").astype(BF).reshape(1, P, PD),
        "pgbc": np.ascontiguousarray(
            np.broadcast_to(g("pair_ln_g")[None], (PD, P, PD))),
        "pbbc": np.ascontiguousarray(
            np.broadcast_to(g("pair_ln_b")[None], (PD, P, PD))),
    }
    in_maps = []
    for c in range(NCORES):
        m = dict(shared)
        m["psT"] = np.ascontiguousarray(psT_full[:, :, c * Bc:(c + 1) * Bc])
        m["msT"] = np.ascontiguousarray(msT_full[:, c * Bc:(c + 1) * Bc])
        in_maps.append(m)
    return in_maps


def _run(inputs, trace=False):
    nc = _get_nc()
    in_maps = _prep_in_maps(inputs)
    res = bass_utils.run_bass_kernel_spmd(
        nc, in_maps, core_ids=list(range(NCORES)), trace=trace
    )
    # out scratch layout [P, Bc//PD, PD, PD] -> (Bc, P, PD) per core
    outp = np.concatenate(
        [
            res.results[c]["out"].transpose(1, 2, 0, 3).reshape(Bc, P, PD)
            for c in range(NCORES)
        ],
        axis=0,
    )
    return np.ascontiguousarray(outp.astype(np.float32)), res


def kernel(**inputs):
    outp, _ = _run(inputs, trace=False)
    return outp



# revision 2
# speedup vs baseline: 2.0920x; 2.0920x over previous
"""Trainium2 Bass kernel for nn_CrossPairMemory.

Sharding: data-parallel over batch across 8 NeuronCores (512 rows each).

Key algebraic restructuring (host-side, exact):
  h_pre = concat(pair_corr, macro_corr) @ w1
        = attn_p @ (vP @ W1p) + attn_m @ (vM @ W1m)
  so the 7168-deep fusion contraction collapses to two 64-deep matmuls
  against precomputed [64, 3584] tables (softmax weights are the only
  device-side activations needed from the memory read).
  Likewise the per-pair output path folds w2 into pair_w:
  out_pre^T[m] = pw0[m]^T ps[m]^T + sum_k Wc[m,k]^T h2[k] + c[m] 1^T
  with Wc[m] = W2[:, m-block] @ pw1[m] and c[m] = b2[m-block] @ pw1[m]
  + pair_b[m], merging the second fusion Linear and the per-pair Linear
  into one 3713-deep accumulation per pair with batch on the free axis.
  Both LayerNorms are column-stat normalizations computed via ones-
  matmuls on the PE + rank-1 broadcasts; gains/biases are per-partition.
"""

import sys

for _p in ("/opt/trn_rl_repo",):
    if _p not in sys.path:
        sys.path.insert(0, _p)

import numpy as np
import ml_dtypes

import concourse.bass as bass
import concourse.tile as tile
from concourse import bacc, mybir
from concourse import bass_utils

BF = ml_dtypes.bfloat16
dt = mybir.dt
AF = mybir.ActivationFunctionType
ALU = mybir.AluOpType

NCORES = 8
B, P, PD, MD, S = 4096, 28, 128, 256, 64
D = P * PD            # 3584
Bc = B // NCORES      # 512 batch rows per core
EPS = 1e-5


def _build(has_c, has_bl2):
    nc = bacc.Bacc(
        "TRN2", target_bir_lowering=False, debug=False, num_devices=NCORES
    )

    def din(name, shape, dty):
        return nc.dram_tensor(name, list(shape), dty, kind="ExternalInput").ap()

    psT = din("psT", (P, PD, Bc), dt.bfloat16)      # pair_states^T per pair
    msT = din("msT", (MD, Bc), dt.bfloat16)         # macro_state^T
    kP = din("kP", (PD, S), dt.bfloat16)            # pair keys^T, pre-scaled
    kM = din("kM", (MD, S), dt.bfloat16)            # macro keys^T, pre-scaled
    vwp = din("vwp", (S, D), dt.bfloat16)           # vP @ W1p
    vwm = din("vwm", (S, D), dt.bfloat16)           # vM @ W1m
    b1t = din("b1t", (PD, P), dt.float32)
    g1t = din("g1t", (PD, P), dt.float32)
    be1t = din("be1t", (PD, P), dt.float32)
    wcr = din("wcr", (P, PD, P, PD), dt.bfloat16)   # [m, kp, kt, e]
    pw0r = din("pw0r", (PD, P, PD), dt.bfloat16)    # [d, m, e]
    cbr = din("cbr", (1, P, PD), dt.bfloat16)       # folded bias rows
    g2t = din("g2t", (PD, P), dt.float32)           # pair_ln_g^T [e, m]
    bl2t = din("bl2t", (PD, P), dt.float32)         # pair_ln_b^T [e, m]
    out = nc.dram_tensor(
        "out", [P, PD, Bc], dt.bfloat16, kind="ExternalOutput"
    ).ap()

    with tile.TileContext(nc) as tc:
        with (
            tc.tile_pool(name="const", bufs=1) as const,
            tc.tile_pool(name="res", bufs=1) as res,
        ):
            # -------- constants + high-priority DMAs (macro path first) ---
            kM0 = const.tile([PD, S], dt.bfloat16, tag="kM0", name="kM0")
            nc.sync.dma_start(kM0, kM[0:PD])
            kM1 = const.tile([PD, S], dt.bfloat16, tag="kM1", name="kM1")
            nc.sync.dma_start(kM1, kM[PD:MD])
            ms0 = const.tile([PD, Bc], dt.bfloat16, tag="ms0", name="ms0")
            nc.sync.dma_start(ms0, msT[0:PD])
            ms1 = const.tile([PD, Bc], dt.bfloat16, tag="ms1", name="ms1")
            nc.sync.dma_start(ms1, msT[PD:MD])
            kP_sb = const.tile([PD, S], dt.bfloat16, tag="kP", name="kP")
            nc.sync.dma_start(kP_sb, kP)
            vwp_sb = res.tile([S, D], dt.bfloat16, tag="vwp", name="vwp")
            nc.sync.dma_start(vwp_sb, vwp)
            vwm_sb = res.tile([S, D], dt.bfloat16, tag="vwm", name="vwm")
            nc.sync.dma_start(vwm_sb, vwm)

            psT_sb = []
            for p in range(P):
                t = res.tile([PD, Bc], dt.bfloat16, tag=f"psT{p}", name=f"psT{p}")
                nc.sync.dma_start(t, psT[p])
                psT_sb.append(t)

            lnc = {}
            for nm, src in (("b1", b1t), ("g1", g1t), ("be1", be1t),
                            ("g2", g2t), ("bl2", bl2t)):
                t = const.tile([PD, P], dt.float32, tag=f"lnc_{nm}", name=f"lnc_{nm}")
                nc.sync.dma_start(t, src)
                lnc[nm] = t
            pw0_sb = const.tile([PD, P, PD], dt.bfloat16, tag="pw0", name="pw0")
            nc.sync.dma_start(pw0_sb, pw0r)
            c_sb = const.tile([1, P, PD], dt.bfloat16, tag="cb", name="cb")
            if has_c:
                nc.sync.dma_start(c_sb, cbr)

            ones_col = const.tile([PD, 1], dt.bfloat16, tag="ones_col", name="ones_col")
            nc.vector.memset(ones_col, 1.0)
            ones_row_f = const.tile([1, PD], dt.float32, tag="ones_row_f", name="ones_row_f")
            nc.vector.memset(ones_row_f, 1.0)
            ones_bc = const.tile([1, Bc], dt.bfloat16, tag="ones_bc", name="ones_bc")
            nc.vector.memset(ones_bc, 1.0)
            eps1 = const.tile([1, 1], dt.float32, tag="eps1", name="eps1")
            nc.vector.memset(eps1, EPS)

            hbuf = [
                res.tile([PD, Bc], dt.bfloat16, tag=f"hb{n}", name=f"hb{n}")
                for n in range(P)
            ]
            ab = {}

            # ---------------- stage A: attention weights ----------------
            with (
                tc.tile_pool(name="stA", bufs=1) as pa,
                tc.tile_pool(name="psA", bufs=2, space="PSUM") as ppa,
            ):
                def softmax_read(which):
                    sp = ppa.tile([S, Bc], dt.float32, tag="sp", name="sp")
                    if which == "pair":
                        for p in range(P):
                            nc.tensor.matmul(
                                sp, kP_sb, psT_sb[p],
                                start=(p == 0), stop=(p == P - 1),
                            )
                    else:
                        nc.tensor.matmul(sp, kM0, ms0, start=True, stop=False)
                        nc.tensor.matmul(sp, kM1, ms1, start=False, stop=True)
                    # scores are O(0.3): exp without max-subtraction is safe
                    eb = pa.tile([S, Bc], dt.bfloat16, tag=f"eb_{which}", name=f"eb_{which}")
                    nc.scalar.activation(eb, sp, AF.Exp)
                    den = ppa.tile([1, Bc], dt.float32, tag="den", name="den")
                    nc.tensor.matmul(den, ones_col[0:S, :], eb, start=True, stop=True)
                    rr = pa.tile([1, Bc], dt.float32, tag=f"rr_{which}", name=f"rr_{which}")
                    nc.vector.reciprocal(rr, den)
                    rbc = ppa.tile([S, Bc], dt.float32, tag="rbc", name="rbc")
                    nc.tensor.matmul(
                        rbc, ones_row_f[:, 0:S], rr, start=True, stop=True
                    )
                    t = res.tile([S, Bc], dt.bfloat16, tag=f"ab_{which}", name=f"ab_{which}")
                    nc.vector.tensor_mul(t, eb, rbc)
                    ab[which] = t

                softmax_read("macro")
                softmax_read("pair")

            # ------------- stage B: folded fusion Linear1 + stats --------
            with tc.tile_pool(name="stat", bufs=1, space="PSUM") as pst:
                stat_h = pst.tile([1, Bc], dt.float32, tag="stat_h", name="stat_h")
                stat_q = pst.tile([1, Bc], dt.float32, tag="stat_q", name="stat_q")

                with (
                    tc.tile_pool(name="psB", bufs=2, space="PSUM") as ppm,
                    tc.tile_pool(name="sqp", bufs=2) as psq,
                ):
                    sq_t = [None] * P

                    def stats_for(n):
                        nc.tensor.matmul(
                            stat_h, ones_col, hbuf[n],
                            start=(n == 0), stop=(n == P - 1),
                            skip_group_check=True,
                        )
                        nc.tensor.matmul(
                            stat_q, ones_col, sq_t[n],
                            start=(n == 0), stop=(n == P - 1),
                            skip_group_check=True,
                        )

                    for n in range(P):
                        pm = ppm.tile([PD, Bc], dt.float32, tag="pm", name="pm")
                        ns = slice(n * PD, (n + 1) * PD)
                        nc.tensor.matmul(
                            pm, vwp_sb[:, ns], ab["pair"], start=True, stop=False
                        )
                        nc.tensor.matmul(
                            pm, vwm_sb[:, ns], ab["macro"], start=False, stop=True
                        )
                        # lag the stats matmuls one iteration so the PE never
                        # waits on the scalar/vector copy of the current tile
                        if n > 0:
                            stats_for(n - 1)
                        nc.scalar.activation(
                            hbuf[n], pm, AF.Identity,
                            bias=lnc["b1"][:, n:n + 1], scale=1.0,
                        )
                        sq = psq.tile([PD, Bc], dt.bfloat16, tag="sq", name="sq")
                        nc.vector.tensor_mul(sq, hbuf[n], hbuf[n])
                        sq_t[n] = sq
                    stats_for(P - 1)

                # ---------------- LayerNorm1 + gelu ----------------------
                with (
                    tc.tile_pool(name="lnrow", bufs=1) as plr,
                    tc.tile_pool(name="psBC", bufs=1, space="PSUM") as ppbc,
                    tc.tile_pool(name="lnb", bufs=1) as plb,
                    tc.tile_pool(name="tnorm", bufs=2) as ptn,
                ):
                    mu_row = plr.tile([1, Bc], dt.float32, tag="mu", name="mu")
                    nc.scalar.activation(mu_row, stat_h, AF.Copy, scale=1.0 / D)
                    m2_row = plr.tile([1, Bc], dt.float32, tag="m2", name="m2")
                    nc.scalar.activation(m2_row, stat_q, AF.Copy, scale=1.0 / D)
                    var_row = plr.tile([1, Bc], dt.float32, tag="va", name="va")
                    nc.vector.tensor_mul(var_row, mu_row, mu_row)
                    nc.vector.tensor_sub(var_row, m2_row, var_row)
                    sd_row = plr.tile([1, Bc], dt.float32, tag="sd", name="sd")
                    nc.scalar.activation(
                        sd_row, var_row, AF.Sqrt, bias=eps1, scale=1.0
                    )
                    rstd_row = plr.tile([1, Bc], dt.float32, tag="rs", name="rs")
                    nc.vector.reciprocal(rstd_row, sd_row)
                    mu_bc = ppbc.tile([PD, Bc], dt.float32, tag="mubc", name="mubc")
                    nc.tensor.matmul(mu_bc, ones_row_f, mu_row, start=True, stop=True)
                    rs_bc = ppbc.tile([PD, Bc], dt.float32, tag="rsbc", name="rsbc")
                    nc.tensor.matmul(rs_bc, ones_row_f, rstd_row, start=True, stop=True)
                    mu_b = plb.tile([PD, Bc], dt.bfloat16, tag="mu_b", name="mu_b")
                    nc.scalar.activation(mu_b, mu_bc, AF.Copy)
                    rs_b = plb.tile([PD, Bc], dt.bfloat16, tag="rs_b", name="rs_b")
                    nc.scalar.activation(rs_b, rs_bc, AF.Copy)

                    for n in range(P):
                        t1 = ptn.tile([PD, Bc], dt.bfloat16, tag="t1", name="t1")
                        nc.vector.scalar_tensor_tensor(
                            t1, hbuf[n], 1.0, mu_b,
                            op0=ALU.mult, op1=ALU.subtract,
                        )
                        t2 = ptn.tile([PD, Bc], dt.bfloat16, tag="t2", name="t2")
                        nc.vector.scalar_tensor_tensor(
                            t2, t1, lnc["g1"][:, n:n + 1], rs_b,
                            op0=ALU.mult, op1=ALU.mult,
                        )
                        # gelu writes h2 back in place over the pre-LN tile
                        nc.scalar.activation(
                            hbuf[n], t2, AF.Gelu,
                            bias=lnc["be1"][:, n:n + 1], scale=1.0,
                        )

            # ------- merged fusion Linear2 + per-pair Linear + LN2 -------
            with (
                tc.tile_pool(name="wcs", bufs=3) as pwc,
                tc.tile_pool(name="psM2", bufs=2, space="PSUM") as ppm2,
                tc.tile_pool(name="st2", bufs=1, space="PSUM") as pst2,
                tc.tile_pool(name="bc2", bufs=1, space="PSUM") as pbc2,
                tc.tile_pool(name="hpo", bufs=3) as php,
                tc.tile_pool(name="rws", bufs=3) as prw,
                tc.tile_pool(name="yout", bufs=3) as pyo,
            ):
                st2h = pst2.tile([1, Bc], dt.float32, tag="st2h", name="st2h")
                st2q = pst2.tile([1, Bc], dt.float32, tag="st2q", name="st2q")
                hpo_t = [None] * P
                rows_t = [None] * P
                bc_t = [None] * P

                def emit_mm(m):
                    wc = pwc.tile([PD, P, PD], dt.bfloat16, tag="wc", name="wc")
                    nc.sync.dma_start(wc, wcr[m])
                    pm2 = ppm2.tile([PD, Bc], dt.float32, tag="pm2", name="pm2")
                    nc.tensor.matmul(
                        pm2, pw0_sb[:, m, :], psT_sb[m], start=True, stop=False
                    )
                    for k in range(P):
                        nc.tensor.matmul(
                            pm2, wc[:, k, :], hbuf[k],
                            start=False, stop=(k == P - 1 and not has_c),
                        )
                    if has_c:
                        nc.tensor.matmul(
                            pm2, c_sb[:, m, :], ones_bc, start=False, stop=True
                        )
                    hpo = php.tile([PD, Bc], dt.bfloat16, tag="hpo", name="hpo")
                    nc.scalar.activation(hpo, pm2, AF.Copy)
                    sq2 = php.tile([PD, Bc], dt.bfloat16, tag="sq2", name="sq2")
                    nc.vector.tensor_mul(sq2, hpo, hpo)
                    hpo_t[m] = (hpo, sq2)

                def emit_stats(m):
                    hpo, sq2 = hpo_t[m]
                    nc.tensor.matmul(
                        st2h, ones_col, hpo, start=True, stop=True,
                        skip_group_check=True,
                    )
                    nc.tensor.matmul(
                        st2q, ones_col, sq2, start=True, stop=True,
                        skip_group_check=True,
                    )
                    mu2 = prw.tile([1, Bc], dt.float32, tag="mu2", name="mu2")
                    nc.scalar.activation(mu2, st2h, AF.Copy, scale=1.0 / PD)
                    m22 = prw.tile([1, Bc], dt.float32, tag="m22", name="m22")
                    nc.scalar.activation(m22, st2q, AF.Copy, scale=1.0 / PD)
                    v2 = prw.tile([1, Bc], dt.float32, tag="v2", name="v2")
                    nc.vector.tensor_mul(v2, mu2, mu2)
                    nc.vector.tensor_sub(v2, m22, v2)
                    sd2 = prw.tile([1, Bc], dt.float32, tag="sd2", name="sd2")
                    nc.scalar.activation(sd2, v2, AF.Sqrt, bias=eps1, scale=1.0)
                    rs2 = prw.tile([1, Bc], dt.float32, tag="rs2", name="rs2")
                    nc.vector.reciprocal(rs2, sd2)
                    rows_t[m] = (mu2, rs2)

                def emit_norm(m):
                    mu2, rs2 = rows_t[m]
                    mu2bc = pbc2.tile([PD, Bc], dt.float32, tag="mu2bc", name="mu2bc")
                    nc.tensor.matmul(mu2bc, ones_row_f, mu2, start=True, stop=True)
                    rs2bc = pbc2.tile([PD, Bc], dt.float32, tag="rs2bc", name="rs2bc")
                    nc.tensor.matmul(rs2bc, ones_row_f, rs2, start=True, stop=True)
                    hpo, _ = hpo_t[m]
                    t1c = pyo.tile([PD, Bc], dt.bfloat16, tag="t1c", name="t1c")
                    nc.vector.scalar_tensor_tensor(
                        t1c, hpo, 1.0, mu2bc, op0=ALU.mult, op1=ALU.subtract
                    )
                    y = pyo.tile([PD, Bc], dt.bfloat16, tag="y", name="y")
                    nc.vector.scalar_tensor_tensor(
                        y, t1c, lnc["g2"][:, m:m + 1], rs2bc,
                        op0=ALU.mult, op1=ALU.mult,
                    )
                    if has_bl2:
                        y2 = pyo.tile([PD, Bc], dt.bfloat16, tag="y2", name="y2")
                        nc.scalar.activation(
                            y2, y, AF.Identity,
                            bias=lnc["bl2"][:, m:m + 1], scale=1.0,
                        )
                        y = y2
                    nc.sync.dma_start(out[m], y)

                # lag-2 schedule: PE never waits on the scalar/vector chain
                for m in range(P):
                    emit_mm(m)
                    if m > 0:
                        emit_stats(m - 1)
                    if m > 1:
                        emit_norm(m - 2)
                emit_stats(P - 1)
                emit_norm(P - 2)
                emit_norm(P - 1)

    nc.compile()
    return nc


_CACHE = {}


def _get_nc(has_c, has_bl2):
    key = (has_c, has_bl2)
    if key not in _CACHE:
        _CACHE[key] = _build(has_c, has_bl2)
    return _CACHE[key]


def _prep(inputs):
    f32 = np.float32
    g = lambda k: np.asarray(inputs[k], f32)

    psT_full = np.asarray(g("pair_states").transpose(1, 2, 0), dtype=BF)   # [P,PD,B]
    msT_full = np.asarray(g("macro_state").T, dtype=BF)                    # [MD,B]

    w1 = g("fusion_w1")
    w2 = g("fusion_w2")
    pw = g("pair_w")
    b2 = g("fusion_b2")
    pb = g("pair_b")

    vwp = (g("mem_pair_vals") @ w1[:D]).astype(BF)                 # (S, D)
    vwm = (g("mem_macro_vals") @ w1[D:]).astype(BF)                # (S, D)
    # Wc[m] = W2[:, m-block] @ pw1[m]  -> [m, kp, kt, e] tiling
    w2b = w2.reshape(D, P, PD)
    wc = np.einsum("dpk,pke->pde", w2b, pw[:, PD:, :])             # (P, D, PD)
    wcr = np.ascontiguousarray(
        wc.reshape(P, P, PD, PD).transpose(0, 2, 1, 3)
    ).astype(BF)                                                   # [m, kp, kt, e]
    cvec = np.einsum("pk,pke->pe", b2.reshape(P, PD), pw[:, PD:, :]) + pb
    has_c = bool(np.abs(cvec).max() > 0)
    bl2 = g("pair_ln_b")
    has_bl2 = bool(np.abs(bl2).max() > 0)

    shared = {
        "kP": np.ascontiguousarray(
            (g("mem_pair_keys").T / (P * np.sqrt(PD))).astype(BF)),
        "kM": np.ascontiguousarray(
            (g("mem_macro_keys").T / np.sqrt(MD)).astype(BF)),
        "vwp": vwp,
        "vwm": vwm,
        "b1t": np.ascontiguousarray(g("fusion_b1").reshape(P, PD).T),
        "g1t": np.ascontiguousarray(g("fusion_ln_g").reshape(P, PD).T),
        "be1t": np.ascontiguousarray(g("fusion_ln_b").reshape(P, PD).T),
        "wcr": wcr,
        "pw0r": np.ascontiguousarray(
            pw[:, :PD, :].transpose(1, 0, 2)).astype(BF),          # [d, m, e]
        "cbr": np.ascontiguousarray(cvec[None]).astype(BF),        # [1, m, e]
        "g2t": np.ascontiguousarray(g("pair_ln_g").T),             # [e, m]
        "bl2t": np.ascontiguousarray(bl2.T),
    }
    in_maps = []
    for c in range(NCORES):
        m = dict(shared)
        m["psT"] = np.ascontiguousarray(psT_full[:, :, c * Bc:(c + 1) * Bc])
        m["msT"] = np.ascontiguousarray(msT_full[:, c * Bc:(c + 1) * Bc])
        in_maps.append(m)
    return in_maps, has_c, has_bl2


def _run(inputs, trace=False):
    in_maps, has_c, has_bl2 = _prep(inputs)
    nc = _get_nc(has_c, has_bl2)
    res = bass_utils.run_bass_kernel_spmd(
        nc, in_maps, core_ids=list(range(NCORES)), trace=trace
    )
    # out [P, PD, Bc] (feature-major) -> (Bc, P, PD) per core
    outp = np.concatenate(
        [
            np.asarray(res.results[c]["out"], np.float32).transpose(2, 0, 1)
            for c in range(NCORES)
        ],
        axis=0,
    )
    return np.ascontiguousarray(outp), res


def kernel(**inputs):
    outp, _ = _run(inputs, trace=False)
    return outp


# revision 4
# speedup vs baseline: 2.5486x; 1.2182x over previous
"""Trainium2 Bass kernel for nn_CrossPairMemory.

Sharding: data-parallel over batch across 8 NeuronCores (512 rows each).

Key algebraic restructuring (host-side, exact):
  h_pre = concat(pair_corr, macro_corr) @ w1
        = attn_p @ (vP @ W1p) + attn_m @ (vM @ W1m)
  so the 7168-deep fusion contraction collapses to two 64-deep matmuls
  against precomputed [64, 3584] tables.  Likewise the per-pair output
  path folds w2 into pair_w:
  out_pre^T[m] = pw0[m]^T ps[m]^T + sum_k Wc[m,k]^T h2[k] + c[m] 1^T
  with Wc[m] = W2[:, m-block] @ pw1[m] and c[m] = b2[m-block] @ pw1[m]
  + pair_b[m], merging the second fusion Linear and the per-pair Linear
  into one accumulation per pair with batch on the free axis.  Both
  LayerNorms are column-stat normalizations via ones-matmuls + rank-1
  broadcasts on the PE; gains are per-partition stt scalars.

Engine balance: row math on GPSIMD, normalize stt alternates
vector/GPSIMD, copies on scalar, reciprocal via the fast custom-DVE
approximation.  Dummy matmuls keep the PE HAM clock-gate warm through
the DMA-bound head and the LayerNorm barrier.
"""

import sys

for _p in ("/opt/trn_rl_repo",):
    if _p not in sys.path:
        sys.path.insert(0, _p)

import numpy as np
import ml_dtypes

import concourse.bass as bass
import concourse.tile as tile
from concourse import bacc, mybir
from concourse import bass_utils

BF = ml_dtypes.bfloat16
dt = mybir.dt
AF = mybir.ActivationFunctionType
ALU = mybir.AluOpType

NCORES = 8
B, P, PD, MD, S = 4096, 28, 128, 256, 64
D = P * PD            # 3584
Bc = B // NCORES      # 512 batch rows per core
EPS = 1e-5


def _build(has_c, has_bl2, has_g2):
    nc = bacc.Bacc(
        "TRN2", target_bir_lowering=False, debug=False, num_devices=NCORES
    )

    def din(name, shape, dty):
        return nc.dram_tensor(name, list(shape), dty, kind="ExternalInput").ap()

    psT = din("psT", (P, PD, Bc), dt.bfloat16)      # pair_states^T per pair
    msT = din("msT", (MD, Bc), dt.bfloat16)         # macro_state^T
    kP = din("kP", (PD, S), dt.bfloat16)            # pair keys^T, pre-scaled
    kM = din("kM", (MD, S), dt.bfloat16)            # macro keys^T, pre-scaled
    vwp = din("vwp", (S, D), dt.bfloat16)           # vP @ W1p
    vwm = din("vwm", (S, D), dt.bfloat16)           # vM @ W1m
    vws = din("vws", (S, 2), dt.bfloat16)           # rowsums for stat_h fold
    b1t = din("b1t", (PD, P), dt.float32)
    g1t = din("g1t", (PD, P), dt.float32)
    be1t = din("be1t", (PD, P), dt.float32)
    wcr = din("wcr", (P, PD, P, PD), dt.bfloat16)   # [m, kp, kt, e]
    pw0r = din("pw0r", (PD, P, PD), dt.bfloat16)    # [d, m, e]
    cbr = din("cbr", (1, P, PD), dt.bfloat16)       # folded bias rows
    g2t = din("g2t", (PD, P), dt.float32)          # pair_ln_g^T [e, m]
    bl2t = din("bl2t", (PD, P), dt.float32)         # pair_ln_b^T [e, m]
    out = nc.dram_tensor(
        "out", [P, PD, Bc], dt.bfloat16, kind="ExternalOutput"
    ).ap()

    with tile.TileContext(nc) as tc:
        with (
            tc.tile_pool(name="const", bufs=1) as const,
            tc.tile_pool(name="res", bufs=1) as res,
            tc.tile_pool(name="psJ", bufs=1, space="PSUM") as psj,
        ):
            # -------- constants + high-priority DMAs (macro path first) ---
            kM0 = const.tile([PD, S], dt.bfloat16, tag="kM0", name="kM0")
            nc.sync.dma_start(kM0, kM[0:PD])
            kM1 = const.tile([PD, S], dt.bfloat16, tag="kM1", name="kM1")
            nc.sync.dma_start(kM1, kM[PD:MD])
            ms0 = const.tile([PD, Bc], dt.bfloat16, tag="ms0", name="ms0")
            nc.sync.dma_start(ms0, msT[0:PD])
            ms1 = const.tile([PD, Bc], dt.bfloat16, tag="ms1", name="ms1")
            nc.sync.dma_start(ms1, msT[PD:MD])
            kP_sb = const.tile([PD, S], dt.bfloat16, tag="kP", name="kP")
            nc.sync.dma_start(kP_sb, kP)

            psT_sb = []
            for p in range(P):
                t = res.tile([PD, Bc], dt.bfloat16, tag=f"psT{p}", name=f"psT{p}")
                nc.sync.dma_start(t, psT[p])
                psT_sb.append(t)

            vwp_sb = res.tile([S, D], dt.bfloat16, tag="vwp", name="vwp")
            nc.sync.dma_start(vwp_sb, vwp)
            vwm_sb = res.tile([S, D], dt.bfloat16, tag="vwm", name="vwm")
            nc.sync.dma_start(vwm_sb, vwm)
            vws_sb = const.tile([S, 2], dt.bfloat16, tag="vws", name="vws")
            nc.sync.dma_start(vws_sb, vws)

            lnc = {}
            for nm, src, dty in (("b1", b1t, dt.float32), ("g1", g1t, dt.float32),
                                 ("be1", be1t, dt.float32), ("g2", g2t, dt.float32),
                                 ("bl2", bl2t, dt.float32)):
                t = const.tile([PD, P], dty, tag=f"lnc_{nm}", name=f"lnc_{nm}")
                nc.sync.dma_start(t, src)
                lnc[nm] = t
            pw0_sb = const.tile([PD, P, PD], dt.bfloat16, tag="pw0", name="pw0")
            nc.sync.dma_start(pw0_sb, pw0r)
            c_sb = const.tile([1, P, PD], dt.bfloat16, tag="cb", name="cb")
            if has_c:
                nc.sync.dma_start(c_sb, cbr)

            ones_col = const.tile([PD, 1], dt.bfloat16, tag="ones_col", name="ones_col")
            nc.vector.memset(ones_col, 1.0)
            ones_row_b = const.tile([1, PD], dt.bfloat16, tag="ones_row_b", name="ones_row_b")
            nc.vector.memset(ones_row_b, 1.0)
            ones_bc = const.tile([1, Bc], dt.bfloat16, tag="ones_bc", name="ones_bc")
            nc.vector.memset(ones_bc, 1.0)
            eps1 = const.tile([1, 1], dt.float32, tag="eps1", name="eps1")
            nc.vector.memset(eps1, EPS)

            # junk psum target for PE HAM-warming dummy matmuls
            junk = psj.tile([S, Bc], dt.float32, tag="junk", name="junk")

            def dummy_mm():
                nc.tensor.matmul(junk, kP_sb, ms0, start=True, stop=True)

            hbuf = [
                res.tile([PD, Bc], dt.bfloat16, tag=f"hb{n}", name=f"hb{n}")
                for n in range(P)
            ]
            ab = {}

            # ---------------- stage A: attention weights ----------------
            with (
                tc.tile_pool(name="stA", bufs=1) as pa,
                tc.tile_pool(name="psA", bufs=2, space="PSUM") as ppa,
            ):
                def softmax_read(which):
                    sp = ppa.tile([S, Bc], dt.float32, tag="sp", name="sp")
                    if which == "pair":
                        for p in range(P):
                            dummy_mm()
                            dummy_mm()
                            nc.tensor.matmul(
                                sp, kP_sb, psT_sb[p],
                                start=(p == 0), stop=(p == P - 1),
                                skip_group_check=True,
                            )
                    else:
                        nc.tensor.matmul(sp, kM0, ms0, start=True, stop=False)
                        nc.tensor.matmul(sp, kM1, ms1, start=False, stop=True)
                    # scores are O(0.3): exp without max-subtraction is safe
                    eb = pa.tile([S, Bc], dt.bfloat16, tag=f"eb_{which}", name=f"eb_{which}")
                    nc.scalar.activation(eb, sp, AF.Exp)
                    den = ppa.tile([1, Bc], dt.float32, tag="den", name="den")
                    nc.tensor.matmul(den, ones_col[0:S, :], eb, start=True, stop=True,
                                     skip_group_check=True)
                    rr = pa.tile([1, Bc], dt.float32, tag=f"rr_{which}", name=f"rr_{which}")
                    nc.vector.reciprocal_approx_fast(rr, den)
                    rrh = pa.tile([1, Bc], dt.bfloat16, tag=f"rrh_{which}", name=f"rrh_{which}")
                    nc.scalar.activation(rrh, rr, AF.Copy)
                    rbc = ppa.tile([S, Bc], dt.float32, tag="rbc", name="rbc")
                    nc.tensor.matmul(
                        rbc, ones_row_b[:, 0:S], rrh, start=True, stop=True,
                        skip_group_check=True,
                    )
                    t = res.tile([S, Bc], dt.bfloat16, tag=f"ab_{which}", name=f"ab_{which}")
                    nc.vector.tensor_mul(t, eb, rbc)
                    ab[which] = t

                softmax_read("macro")
                softmax_read("pair")

            # ------------- stage B: folded fusion Linear1 + stats --------
            with tc.tile_pool(name="stat", bufs=1, space="PSUM") as pst:
                stat_h = pst.tile([1, Bc], dt.float32, tag="stat_h", name="stat_h")
                stat_q = pst.tile([1, Bc], dt.float32, tag="stat_q", name="stat_q")

                with (
                    tc.tile_pool(name="psB", bufs=2, space="PSUM") as ppm,
                    tc.tile_pool(name="sqp", bufs=2) as psq,
                ):
                    sq_t = [None] * P

                    # stat_h folds through the linear algebra: softmax rows
                    # sum to 1, so sum_f h = ab_p @ rowsum(VWp + b1) +
                    # ab_m @ rowsum(VWm)  (host-folded into vws)
                    nc.tensor.matmul(
                        stat_h, vws_sb[:, 0:1], ab["pair"],
                        start=True, stop=False, skip_group_check=True,
                    )
                    nc.tensor.matmul(
                        stat_h, vws_sb[:, 1:2], ab["macro"],
                        start=False, stop=True, skip_group_check=True,
                    )

                    def stats_for(n):
                        nc.tensor.matmul(
                            stat_q, ones_col, sq_t[n],
                            start=(n == 0), stop=(n == P - 1),
                            skip_group_check=True,
                        )

                    for n in range(P):
                        pm = ppm.tile([PD, Bc], dt.float32, tag="pm", name="pm")
                        ns = slice(n * PD, (n + 1) * PD)
                        nc.tensor.matmul(
                            pm, vwp_sb[:, ns], ab["pair"], start=True, stop=False
                        )
                        nc.tensor.matmul(
                            pm, vwm_sb[:, ns], ab["macro"], start=False, stop=True
                        )
                        # lag the stats matmuls one iteration so the PE never
                        # waits on the copy of the current tile
                        if n > 0:
                            stats_for(n - 1)
                        if n % 2 == 0:
                            nc.scalar.activation(
                                hbuf[n], pm, AF.Identity,
                                bias=lnc["b1"][:, n:n + 1], scale=1.0,
                            )
                        else:
                            nc.vector.tensor_scalar_add(
                                hbuf[n], pm, lnc["b1"][:, n:n + 1]
                            )
                        sq = psq.tile([PD, Bc], dt.bfloat16, tag="sq", name="sq")
                        nc.gpsimd.tensor_mul(sq, hbuf[n], hbuf[n])
                        sq_t[n] = sq
                    stats_for(P - 1)

                # ---------------- LayerNorm1 + gelu ----------------------
                with (
                    tc.tile_pool(name="lnrow", bufs=1) as plr,
                    tc.tile_pool(name="psBC", bufs=1, space="PSUM") as ppbc,
                    tc.tile_pool(name="lnb", bufs=1) as plb,
                    tc.tile_pool(name="tnorm", bufs=2) as ptn,
                ):
                    mu_h = plr.tile([1, Bc], dt.bfloat16, tag="muh", name="muh")
                    nc.scalar.activation(mu_h, stat_h, AF.Copy, scale=1.0 / D)
                    m2_row = plr.tile([1, Bc], dt.float32, tag="m2", name="m2")
                    nc.scalar.activation(m2_row, stat_q, AF.Copy, scale=1.0 / D)
                    var_row = plr.tile([1, Bc], dt.float32, tag="va", name="va")
                    nc.gpsimd.tensor_mul(var_row, mu_h, mu_h)
                    nc.gpsimd.tensor_sub(var_row, m2_row, var_row)
                    sd_row = plr.tile([1, Bc], dt.float32, tag="sd", name="sd")
                    nc.scalar.activation(
                        sd_row, var_row, AF.Sqrt, bias=eps1, scale=1.0
                    )
                    rstd_row = plr.tile([1, Bc], dt.float32, tag="rs", name="rs")
                    nc.vector.reciprocal_approx_fast(rstd_row, sd_row)
                    rstd_h = plr.tile([1, Bc], dt.bfloat16, tag="rsh", name="rsh")
                    nc.scalar.activation(rstd_h, rstd_row, AF.Copy)
                    mu_bc = ppbc.tile([PD, Bc], dt.float32, tag="mubc", name="mubc")
                    nc.tensor.matmul(mu_bc, ones_row_b, mu_h, start=True, stop=True)
                    rs_bc = ppbc.tile([PD, Bc], dt.float32, tag="rsbc", name="rsbc")
                    nc.tensor.matmul(rs_bc, ones_row_b, rstd_h, start=True, stop=True)
                    mu_b = plb.tile([PD, Bc], dt.bfloat16, tag="mu_b", name="mu_b")
                    nc.scalar.activation(mu_b, mu_bc, AF.Copy)
                    rs_b = plb.tile([PD, Bc], dt.bfloat16, tag="rs_b", name="rs_b")
                    nc.scalar.activation(rs_b, rs_bc, AF.Copy)

                    for n in range(P):
                        eng = nc.vector if n % 2 == 0 else nc.gpsimd
                        t1 = ptn.tile([PD, Bc], dt.bfloat16, tag=f"t1{n % 2}", name=f"t1{n % 2}")
                        eng.tensor_sub(t1, hbuf[n], mu_b)
                        t2 = ptn.tile([PD, Bc], dt.bfloat16, tag=f"t2{n % 2}", name=f"t2{n % 2}")
                        eng.tensor_mul(t2, t1, rs_b)
                        dummy_mm()
                        # gelu(t2 * g + b): gain rides the activation scale
                        nc.scalar.activation(
                            hbuf[n], t2, AF.Gelu,
                            bias=lnc["be1"][:, n:n + 1],
                            scale=lnc["g1"][:, n:n + 1],
                        )

            # ------- merged fusion Linear2 + per-pair Linear + LN2 -------
            with (
                tc.tile_pool(name="wcs", bufs=3) as pwc,
                tc.tile_pool(name="psM2", bufs=2, space="PSUM") as ppm2,
                tc.tile_pool(name="st2", bufs=1, space="PSUM") as pst2,
                tc.tile_pool(name="bc2", bufs=1, space="PSUM") as pbc2,
                tc.tile_pool(name="hpo", bufs=3) as php,
                tc.tile_pool(name="rws", bufs=3) as prw,
                tc.tile_pool(name="yout", bufs=3) as pyo,
            ):
                st2h = pst2.tile([1, Bc], dt.float32, tag="st2h", name="st2h")
                st2q = pst2.tile([1, Bc], dt.float32, tag="st2q", name="st2q")
                hpo_t = [None] * P
                rows_t = [None] * P

                def emit_mm(m):
                    wc = pwc.tile([PD, P, PD], dt.bfloat16, tag="wc", name="wc")
                    nc.sync.dma_start(wc, wcr[m])
                    pm2 = ppm2.tile([PD, Bc], dt.float32, tag="pm2", name="pm2")
                    nc.tensor.matmul(
                        pm2, pw0_sb[:, m, :], psT_sb[m], start=True, stop=False
                    )
                    for k in range(P):
                        nc.tensor.matmul(
                            pm2, wc[:, k, :], hbuf[k],
                            start=False, stop=(k == P - 1 and not has_c),
                        )
                    if has_c:
                        nc.tensor.matmul(
                            pm2, c_sb[:, m, :], ones_bc, start=False, stop=True
                        )
                    hpo = php.tile([PD, Bc], dt.bfloat16, tag="hpo", name="hpo")
                    nc.scalar.activation(hpo, pm2, AF.Copy)
                    sq2 = php.tile([PD, Bc], dt.bfloat16, tag="sq2", name="sq2")
                    nc.vector.tensor_mul(sq2, hpo, hpo)
                    hpo_t[m] = (hpo, sq2)

                def emit_stats(m):
                    hpo, sq2 = hpo_t[m]
                    nc.tensor.matmul(
                        st2h, ones_col, hpo, start=True, stop=True,
                        skip_group_check=True,
                    )
                    nc.tensor.matmul(
                        st2q, ones_col, sq2, start=True, stop=True,
                        skip_group_check=True,
                    )
                    mu2 = prw.tile([1, Bc], dt.bfloat16, tag="mu2", name="mu2")
                    nc.scalar.activation(mu2, st2h, AF.Copy, scale=1.0 / PD)
                    m22 = prw.tile([1, Bc], dt.float32, tag="m22", name="m22")
                    nc.scalar.activation(m22, st2q, AF.Copy, scale=1.0 / PD)
                    v2 = prw.tile([1, Bc], dt.float32, tag="v2", name="v2")
                    nc.gpsimd.tensor_mul(v2, mu2, mu2)
                    nc.gpsimd.tensor_sub(v2, m22, v2)
                    sd2 = prw.tile([1, Bc], dt.float32, tag="sd2", name="sd2")
                    nc.scalar.activation(sd2, v2, AF.Sqrt, bias=eps1, scale=1.0)
                    rs2 = prw.tile([1, Bc], dt.float32, tag="rs2", name="rs2")
                    nc.vector.reciprocal_approx_fast(rs2, sd2)
                    rs2h = prw.tile([1, Bc], dt.bfloat16, tag="rs2h", name="rs2h")
                    nc.scalar.activation(rs2h, rs2, AF.Copy)
                    rows_t[m] = (mu2, rs2h)

                def emit_norm(m):
                    mu2, rs2h = rows_t[m]
                    mu2bc = pbc2.tile([PD, Bc], dt.float32, tag="mu2bc", name="mu2bc")
                    nc.tensor.matmul(mu2bc, ones_row_b, mu2, start=True, stop=True)
                    rs2bc = pbc2.tile([PD, Bc], dt.float32, tag="rs2bc", name="rs2bc")
                    nc.tensor.matmul(rs2bc, ones_row_b, rs2h, start=True, stop=True)
                    mu2b = pyo.tile([PD, Bc], dt.bfloat16, tag="mu2b", name="mu2b")
                    nc.scalar.activation(mu2b, mu2bc, AF.Copy)
                    rs2b = pyo.tile([PD, Bc], dt.bfloat16, tag="rs2b", name="rs2b")
                    nc.scalar.activation(rs2b, rs2bc, AF.Copy)
                    hpo, _ = hpo_t[m]
                    t1c = pyo.tile([PD, Bc], dt.bfloat16, tag="t1c", name="t1c")
                    nc.vector.tensor_sub(t1c, hpo, mu2b)
                    y = pyo.tile([PD, Bc], dt.bfloat16, tag="y", name="y")
                    nc.gpsimd.tensor_mul(y, t1c, rs2b)
                    if has_g2 or has_bl2:
                        y2 = pyo.tile([PD, Bc], dt.bfloat16, tag="y2", name="y2")
                        nc.scalar.activation(
                            y2, y, AF.Identity,
                            bias=lnc["bl2"][:, m:m + 1],
                            scale=lnc["g2"][:, m:m + 1],
                        )
                        y = y2
                    nc.sync.dma_start(out[m], y)

                # lag-2 schedule: PE never waits on the scalar/vector chain
                for m in range(P):
                    emit_mm(m)
                    if m > 0:
                        emit_stats(m - 1)
                    if m > 1:
                        emit_norm(m - 2)
                emit_stats(P - 1)
                emit_norm(P - 2)
                emit_norm(P - 1)

    nc.compile()
    return nc


_CACHE = {}


def _get_nc(has_c, has_bl2, has_g2):
    key = (has_c, has_bl2, has_g2)
    if key not in _CACHE:
        _CACHE[key] = _build(has_c, has_bl2, has_g2)
    return _CACHE[key]


def _prep(inputs):
    f32 = np.float32
    g = lambda k: np.asarray(inputs[k], f32)

    psT_full = np.asarray(g("pair_states").transpose(1, 2, 0), dtype=BF)   # [P,PD,B]
    msT_full = np.asarray(g("macro_state").T, dtype=BF)                    # [MD,B]

    w1 = g("fusion_w1")
    w2 = g("fusion_w2")
    pw = g("pair_w")
    b2 = g("fusion_b2")
    pb = g("pair_b")

    vwp = (g("mem_pair_vals") @ w1[:D]).astype(BF)                 # (S, D)
    vwm = (g("mem_macro_vals") @ w1[D:]).astype(BF)                # (S, D)
    # Wc[m] = W2[:, m-block] @ pw1[m]  -> [m, kp, kt, e] tiling
    w2b = w2.reshape(D, P, PD)
    wc = np.einsum("dpk,pke->pde", w2b, pw[:, PD:, :])             # (P, D, PD)
    wcr = np.ascontiguousarray(
        wc.reshape(P, P, PD, PD).transpose(0, 2, 1, 3)
    ).astype(BF)                                                   # [m, kp, kt, e]
    cvec = np.einsum("pk,pke->pe", b2.reshape(P, PD), pw[:, PD:, :]) + pb
    has_c = bool(np.abs(cvec).max() > 0)
    bl2 = g("pair_ln_b")
    has_bl2 = bool(np.abs(bl2).max() > 0)
    has_g2 = bool(np.abs(g("pair_ln_g") - 1.0).max() > 0)
    vws = np.stack([
        g("mem_pair_vals") @ (w1[:D].sum(1) + g("fusion_b1").sum()),
        g("mem_macro_vals") @ w1[D:].sum(1),
    ], axis=1).astype(BF)                                          # (S, 2)

    shared = {
        "kP": np.ascontiguousarray(
            (g("mem_pair_keys").T / (P * np.sqrt(PD))).astype(BF)),
        "kM": np.ascontiguousarray(
            (g("mem_macro_keys").T / np.sqrt(MD)).astype(BF)),
        "vwp": vwp,
        "vwm": vwm,
        "vws": vws,
        "b1t": np.ascontiguousarray(g("fusion_b1").reshape(P, PD).T),
        "g1t": np.ascontiguousarray(g("fusion_ln_g").reshape(P, PD).T),
        "be1t": np.ascontiguousarray(g("fusion_ln_b").reshape(P, PD).T),
        "wcr": wcr,
        "pw0r": np.ascontiguousarray(
            pw[:, :PD, :].transpose(1, 0, 2)).astype(BF),          # [d, m, e]
        "cbr": np.ascontiguousarray(cvec[None]).astype(BF),        # [1, m, e]
        "g2t": np.ascontiguousarray(g("pair_ln_g").T),             # [e, m]
        "bl2t": np.ascontiguousarray(bl2.T),
    }
    in_maps = []
    for c in range(NCORES):
        m = dict(shared)
        m["psT"] = np.ascontiguousarray(psT_full[:, :, c * Bc:(c + 1) * Bc])
        m["msT"] = np.ascontiguousarray(msT_full[:, c * Bc:(c + 1) * Bc])
        in_maps.append(m)
    return in_maps, (has_c, has_bl2, has_g2)


def _run(inputs, trace=False):
    in_maps, flags = _prep(inputs)
    nc = _get_nc(*flags)
    res = bass_utils.run_bass_kernel_spmd(
        nc, in_maps, core_ids=list(range(NCORES)), trace=trace
    )
    # out [P, PD, Bc] (feature-major) -> (Bc, P, PD) per core
    outp = np.concatenate(
        [
            np.asarray(res.results[c]["out"], np.float32).transpose(2, 0, 1)
            for c in range(NCORES)
        ],
        axis=0,
    )
    return np.ascontiguousarray(outp), res


def kernel(**inputs):
    outp, _ = _run(inputs, trace=False)
    return outp
